# revision 17
# baseline (speedup 1.0000x reference)
"""BitFeedForward (Hadamard + int8 act-quant + ternary weights) on 8 TRN2 cores.

Data-parallel over tokens (8192 tokens -> 1024 per core, 4 blocks of 256).
Weights are ternarized on the host (static packing: mean-abs scale + ternary
cast to fp8, transposed) so the device runs only the per-token path:
  FWHT = H128 on the PE (fp32r) + butterfly stages on DVE/GPSIMD (stage 1
  fused into the PSUM eviction), act quant -> int8-valued bf16 (exact),
  GEMMs bf16 x fp8 on the PE with exact integer arithmetic, per-token
  scales applied analytically at the end.
Weight matrices stream from DRAM in double-buffered slices; emission is a
2-stage software pipeline (layer-1+GEMM1 of block b+1 ahead of
layer-2+GEMM2 of block b) to keep the PE dense across the butterfly chains.
"""
import math
import numpy as np
import ml_dtypes
from contextlib import ExitStack

import concourse.bass as bass
from concourse import bacc
import concourse.tile as tile
import concourse.mybir as mybir
from concourse.bass_utils import run_bass_kernel_spmd
from concourse.masks import make_identity

F32 = mybir.dt.float32
F32R = mybir.dt.float32r
BF16 = mybir.dt.bfloat16
FP8 = mybir.dt.float8e4
NP_FP8 = ml_dtypes.float8_e4m3

NCORES = 8
B, S, H, I = 4, 2048, 2048, 4096
TOKENS = B * S            # 8192
T = TOKENS // NCORES      # 1024 tokens per core
TB = 256                  # tokens per block
NB = T // TB              # 4 blocks
NC1 = H // 128            # 16 k-chunks for layer 1
NC2 = I // 128            # 32 chunks for layer 2
CM = 12582912.0           # 1.5 * 2**23: fp32 add/sub rounds to nearest int
ISQ1 = 1.0 / math.sqrt(H)

ADD = mybir.AluOpType.add
SUB = mybir.AluOpType.subtract
MULT = mybir.AluOpType.mult
MAX = mybir.AluOpType.max
AF = mybir.ActivationFunctionType


def _bfly(eng, out_t, in_t, nchunk, sigma, span):
    """One FWHT butterfly stage over the chunk axis of [128, nchunk*span]."""
    iv = in_t[:].rearrange("p (g two s) -> p g two s", two=2, s=sigma * span)
    ov = out_t[:].rearrange("p (g two s) -> p g two s", two=2, s=sigma * span)
    assert iv.shape[1] == nchunk // (2 * sigma)
    eng.tensor_tensor(ov[:, :, 0, :], iv[:, :, 0, :], iv[:, :, 1, :], ADD)
    eng.tensor_tensor(ov[:, :, 1, :], iv[:, :, 0, :], iv[:, :, 1, :], SUB)


def build():
    nc = bacc.Bacc()
    x_in = nc.declare_dram_parameter("xT", [H, T], F32, isOutput=False)
    wu_in = nc.declare_dram_parameter("wu", [H, I], FP8, isOutput=False)
    wd_in = nc.declare_dram_parameter("wd", [I, H], FP8, isOutput=False)
    ws_in = nc.declare_dram_parameter("ws", [1, 2], F32, isOutput=False)
    h128_in = nc.declare_dram_parameter("h128", [128, 128], F32, isOutput=False)
    out_d = nc.declare_dram_parameter("out", [T, H], F32, isOutput=True)

    with tile.TileContext(nc) as tc, ExitStack() as ctx:
        const = ctx.enter_context(tc.tile_pool(name="const", bufs=1))
        t1p = ctx.enter_context(tc.tile_pool(name="t1p", bufs=2))
        t2p = ctx.enter_context(tc.tile_pool(name="t2p", bufs=2))
        xp = ctx.enter_context(tc.tile_pool(name="xp", bufs=2))
        l1p = ctx.enter_context(tc.tile_pool(name="l1p", bufs=1))
        q1p = ctx.enter_context(tc.tile_pool(name="q1p", bufs=1))
        rP = ctx.enter_context(tc.tile_pool(name="rP", bufs=1))
        l2p = ctx.enter_context(tc.tile_pool(name="l2p", bufs=1))
        q2p = ctx.enter_context(tc.tile_pool(name="q2p", bufs=1))
        rpp = ctx.enter_context(tc.tile_pool(name="rpp", bufs=1))
        shp = ctx.enter_context(tc.tile_pool(name="shp", bufs=2))
        outp = ctx.enter_context(tc.tile_pool(name="outp", bufs=1))
        med = ctx.enter_context(tc.tile_pool(name="med", bufs=1))
        tiny = ctx.enter_context(tc.tile_pool(name="tiny", bufs=1))
        tiny2 = ctx.enter_context(tc.tile_pool(name="tiny2", bufs=2))
        ps_h = ctx.enter_context(tc.tile_pool(name="ps_h", bufs=2, space="PSUM"))
        ps_a1 = ctx.enter_context(tc.tile_pool(name="ps_a1", bufs=2, space="PSUM"))
        ps_a2 = ctx.enter_context(tc.tile_pool(name="ps_a2", bufs=2, space="PSUM"))
        ps_sm = ctx.enter_context(tc.tile_pool(name="ps_sm", bufs=1, space="PSUM"))

        ident = const.tile([128, 128], F32)
        make_identity(nc, ident[:])
        h128 = const.tile([128, 128], F32)
        nc.sync.dma_start(h128[:], h128_in[:])
        h128r = const.tile([128, 128], F32R)
        nc.vector.tensor_copy(h128r[:], h128[:])
        ones_row = const.tile([1, 128], F32)
        nc.vector.memset(ones_row[:], 1.0)

        # broadcast [s1, s2] across partitions: sW[p, j] = s_j
        srow = const.tile([1, 2], F32)
        nc.sync.dma_start(srow[:], ws_in[:])
        psb = ps_sm.tile([128, 2], F32, tag="sm")
        nc.tensor.matmul(psb[:], ones_row[:], srow[:], start=True, stop=True)
        sW = const.tile([128, 2], F32)
        nc.vector.tensor_copy(sW[:], psb[:])

        st = [dict() for _ in range(NB)]  # per-block cross-phase tiles

        def h2part(bi):
            """H128 pass of layer 2 for block bi + fused sigma=1 butterfly."""
            d = st[bi]
            r = d["r"]
            vA = l2p.tile([128, NC2 * TB], F32, tag="ping")
            d["vA"] = vA
            for g in range(NC2 // 2):
                ph = ps_h.tile([128, 2 * TB], F32, tag="ph")
                for k in range(2):
                    m = 2 * g + k
                    nc.tensor.matmul(
                        ph[:, k * TB:(k + 1) * TB], h128r[:],
                        r[:, m * TB:(m + 1) * TB], start=True, stop=True)
                sh = shp.tile([128, TB], F32, tag="sh")
                nc.scalar.copy(sh[:], ph[:, 0:TB])
                nc.vector.tensor_tensor(
                    vA[:, (2 * g) * TB:(2 * g + 1) * TB],
                    sh[:], ph[:, TB:2 * TB], ADD)
                nc.vector.tensor_tensor(
                    vA[:, (2 * g + 1) * TB:(2 * g + 2) * TB],
                    sh[:], ph[:, TB:2 * TB], SUB)

        def front(bi):
            """x load, layer-1 FWHT + quant, GEMM1, relu^2 -> r (f32r)."""
            d = st[bi]
            tok0 = bi * TB

            # H128 pass of layer 1 with fused sigma=1 butterfly
            fA = l1p.tile([128, NC1 * TB], F32, tag="lp1")
            for g in range(NC1 // 2):
                xt = xp.tile([128, 2 * TB], F32, tag="x")
                nc.sync.dma_start(
                    xt[:].rearrange("p (c t) -> p c t", c=2),
                    x_in[g * 256:(g + 1) * 256, tok0:tok0 + TB].rearrange(
                        "(c p) t -> p c t", p=128),
                )
                ph = ps_h.tile([128, 2 * TB], F32, tag="ph")
                for k in range(2):
                    nc.tensor.matmul(
                        ph[:, k * TB:(k + 1) * TB], h128[:],
                        xt[:, k * TB:(k + 1) * TB], start=True, stop=True)
                sh = shp.tile([128, TB], F32, tag="sh")
                nc.scalar.copy(sh[:], ph[:, 0:TB])
                nc.vector.tensor_tensor(
                    fA[:, (2 * g) * TB:(2 * g + 1) * TB],
                    sh[:], ph[:, TB:2 * TB], ADD)
                nc.vector.tensor_tensor(
                    fA[:, (2 * g + 1) * TB:(2 * g + 2) * TB],
                    sh[:], ph[:, TB:2 * TB], SUB)
            fB = l1p.tile([128, NC1 * TB], F32, tag="lp2")
            _bfly(nc.gpsimd, fB, fA, NC1, 2, TB)
            fC = l1p.tile([128, NC1 * TB], F32, tag="lp1")
            _bfly(nc.vector, fC, fB, NC1, 4, TB)
            fD = l1p.tile([128, NC1 * TB], F32, tag="lp2")
            _bfly(nc.gpsimd, fD, fC, NC1, 8, TB)

            # per-token absmax -> scales (layer 1)
            P1 = med.tile([128, TB], F32, tag="p1")
            nc.vector.tensor_reduce(
                P1[:], fD[:].rearrange("p (c t) -> p t c", c=NC1),
                mybir.AxisListType.X, MAX, apply_absolute_value=True)
            pjt = ps_sm.tile([128, TB], F32, tag="sm")
            for j in range(2):
                nc.tensor.transpose(
                    pjt[:, j * 128:(j + 1) * 128], P1[:, j * 128:(j + 1) * 128],
                    ident[:])
            Mu = tiny.tile([128, 2], F32, tag="mu1")
            for j in range(2):
                nc.vector.tensor_reduce(
                    Mu[:, j:j + 1], pjt[:, j * 128:(j + 1) * 128],
                    mybir.AxisListType.X, MAX)
            M1 = tiny.tile([128, 2], F32, tag="m1")
            nc.vector.tensor_scalar(M1[:], Mu[:], ISQ1, 1e-5, MULT, MAX)
            rM1 = tiny.tile([128, 2], F32, tag="rm1")
            nc.vector.reciprocal(rM1[:], M1[:])
            s1t = tiny.tile([128, 2], F32, tag="s1t")
            nc.vector.tensor_scalar(s1t[:], rM1[:], 127.0 * ISQ1, None, MULT)
            ct = tiny.tile([128, 2], F32, tag="ct")
            nc.vector.tensor_tensor(ct[:], M1[:], sW[:, 0:1].broadcast_to([128, 2]), MULT)
            cc = tiny2.tile([128, 2], F32, tag="cc")
            nc.vector.tensor_tensor(cc[:], ct[:], ct[:], MULT)
            nc.vector.tensor_scalar(cc[:], cc[:], 1.0 / (127.0 * 127.0 * 64.0), None, MULT)
            d["cc"] = cc

            rows1 = []
            for j in range(2):
                stp = ps_sm.tile([1, 128], F32, tag="sm")
                nc.tensor.transpose(stp[:], s1t[:, j:j + 1], ident[:])
                rj = tiny.tile([1, 128], F32, tag=f"r1{j}")
                nc.vector.tensor_copy(rj[:], stp[:])
                rows1.append(rj)
            sbp = ps_sm.tile([128, TB], F32, tag="sm")
            for j in range(2):
                nc.tensor.matmul(
                    sbp[:, j * 128:(j + 1) * 128], ones_row[:], rows1[j][:],
                    start=True, stop=True)
            S1B = med.tile([128, TB], F32, tag="s1b")
            nc.vector.tensor_copy(S1B[:], sbp[:])

            # q1 = round(u * s1t), int8-valued bf16
            um = l1p.tile([128, NC1 * TB], F32, tag="lp1")
            nc.vector.tensor_tensor(
                um[:].rearrange("p (c t) -> p c t", c=NC1),
                fD[:].rearrange("p (c t) -> p c t", c=NC1),
                S1B[:, None, :].broadcast_to([128, NC1, TB]), MULT)
            q1t = q1p.tile([128, NC1 * TB], BF16, tag="q1")
            nc.vector.tensor_scalar(q1t[:], um[:], CM, CM, ADD, SUB)

            # GEMM1 + ReLU^2, streaming w_up^T in 512-col slices
            r = rP.tile([128, NC2 * TB], F32R, tag="r")
            d["r"] = r
            for s in range(I // 512):
                t1s = t1p.tile([128, NC1 * 512], FP8, tag="t1")
                nc.sync.dma_start(
                    t1s[:].rearrange("p (c o) -> p c o", c=NC1),
                    wu_in[:, s * 512:(s + 1) * 512].rearrange(
                        "(c p) o -> p c o", p=128),
                )
                for ocp in range(2):
                    acc = ps_a1.tile([128, 512], F32, tag="a1")
                    for half in range(2):
                        oc = ocp * 2 + half
                        for cp in range(NC1):
                            nc.tensor.matmul(
                                acc[:, half * TB:(half + 1) * TB],
                                t1s[:, cp * 512 + oc * 128: cp * 512 + (oc + 1) * 128],
                                q1t[:, cp * TB:(cp + 1) * TB],
                                start=(cp == 0), stop=(cp == NC1 - 1))
                    rp = rpp.tile([128, 512], F32, tag="rp")
                    nc.scalar.activation(rp[:], acc[:], AF.Relu, bias=0.0, scale=1.0)
                    m0 = s * 4 + ocp * 2
                    nc.scalar.activation(
                        r[:, m0 * TB:(m0 + 2) * TB], rp[:], AF.Square, bias=0.0)

        def back(bi):
            """layer-2 butterflies + quant, GEMM2, output."""
            d = st[bi]
            tok0 = bi * TB
            cc = d["cc"]
            vA = d["vA"]

            vB = l2p.tile([128, NC2 * TB], F32, tag="pong")
            _bfly(nc.gpsimd, vB, vA, NC2, 2, TB)
            vC = l2p.tile([128, NC2 * TB], F32, tag="ping")
            _bfly(nc.vector, vC, vB, NC2, 4, TB)
            vD = l2p.tile([128, NC2 * TB], F32, tag="pong")
            _bfly(nc.gpsimd, vD, vC, NC2, 8, TB)
            vE = l2p.tile([128, NC2 * TB], F32, tag="ping")
            _bfly(nc.vector, vE, vD, NC2, 16, TB)

            P2 = med.tile([128, TB], F32, tag="p2")
            nc.vector.tensor_reduce(
                P2[:], vE[:].rearrange("p (m t) -> p t m", m=NC2),
                mybir.AxisListType.X, MAX, apply_absolute_value=True)
            pjt2 = ps_sm.tile([128, TB], F32, tag="sm")
            for j in range(2):
                nc.tensor.transpose(
                    pjt2[:, j * 128:(j + 1) * 128], P2[:, j * 128:(j + 1) * 128],
                    ident[:])
            Mu2 = tiny.tile([128, 2], F32, tag="mu2")
            for j in range(2):
                nc.vector.tensor_reduce(
                    Mu2[:, j:j + 1], pjt2[:, j * 128:(j + 1) * 128],
                    mybir.AxisListType.X, MAX)
            M2 = tiny.tile([128, 2], F32, tag="m2")
            nc.vector.tensor_tensor(M2[:], Mu2[:], cc[:], MULT)
            nc.vector.tensor_scalar(M2[:], M2[:], 1e-5, None, MAX)
            rM2 = tiny.tile([128, 2], F32, tag="rm2")
            nc.vector.reciprocal(rM2[:], M2[:])
            s2t = tiny.tile([128, 2], F32, tag="s2t")
            nc.vector.tensor_tensor(s2t[:], rM2[:], cc[:], MULT)
            nc.vector.tensor_scalar(s2t[:], s2t[:], 127.0, None, MULT)
            f = tiny.tile([128, 2], F32, tag="f")
            nc.vector.tensor_tensor(f[:], M2[:], sW[:, 1:2].broadcast_to([128, 2]), MULT)
            nc.vector.tensor_scalar(f[:], f[:], 1.0 / 127.0, None, MULT)

            rows2 = []
            for j in range(2):
                stp = ps_sm.tile([1, 128], F32, tag="sm")
                nc.tensor.transpose(stp[:], s2t[:, j:j + 1], ident[:])
                rj = tiny.tile([1, 128], F32, tag=f"r2{j}")
                nc.vector.tensor_copy(rj[:], stp[:])
                rows2.append(rj)
            sbp2 = ps_sm.tile([128, TB], F32, tag="sm")
            for j in range(2):
                nc.tensor.matmul(
                    sbp2[:, j * 128:(j + 1) * 128], ones_row[:], rows2[j][:],
                    start=True, stop=True)
            S2B = med.tile([128, TB], F32, tag="s2b")
            nc.vector.tensor_copy(S2B[:], sbp2[:])

            # vm + round, split by token halves so GEMM2 j=0 starts early
            vm = l2p.tile([128, NC2 * TB], F32, tag="pong")
            q2t = q2p.tile([128, NC2 * TB], BF16, tag="q2")
            for j in range(2):
                tj = slice(j * 128, (j + 1) * 128)
                nc.vector.tensor_tensor(
                    vm[:].rearrange("p (m t) -> p m t", m=NC2)[:, :, tj],
                    vE[:].rearrange("p (m t) -> p m t", m=NC2)[:, :, tj],
                    S2B[:, None, tj].broadcast_to([128, NC2, 128]), MULT)
                nc.vector.tensor_scalar(
                    q2t[:].rearrange("p (m t) -> p m t", m=NC2)[:, :, tj],
                    vm[:].rearrange("p (m t) -> p m t", m=NC2)[:, :, tj],
                    CM, CM, ADD, SUB)

            # GEMM2 tokens-stationary, streaming w_down^T in 512-col slices
            for hs in range(4):
                halves = []
                for mh in range(2):
                    t2s = t2p.tile([128, 16 * 512], FP8, tag="t2")
                    nc.sync.dma_start(
                        t2s[:].rearrange("p (m h) -> p m h", m=16),
                        wd_in[mh * 2048:(mh + 1) * 2048,
                              hs * 512:(hs + 1) * 512].rearrange(
                            "(m p) h -> p m h", p=128),
                    )
                    halves.append(t2s)
                for j in range(2):
                    acc2 = ps_a2.tile([128, 512], F32, tag="a2")
                    for m2 in range(NC2):
                        t2s = halves[m2 // 16]
                        nc.tensor.matmul(
                            acc2[:],
                            q2t[:, m2 * TB + j * 128: m2 * TB + (j + 1) * 128],
                            t2s[:, (m2 % 16) * 512:((m2 % 16) + 1) * 512],
                            start=(m2 == 0), stop=(m2 == NC2 - 1))
                    ot = outp.tile([128, 512], F32, tag="ot")
                    nc.scalar.activation(
                        ot[:], acc2[:], AF.Identity, bias=0.0, scale=f[:, j:j + 1])
                    nc.sync.dma_start(
                        out_d[tok0 + j * 128: tok0 + (j + 1) * 128,
                              hs * 512:(hs + 1) * 512], ot[:])

        for bi in range(NB):
            if bi >= 1:
                h2part(bi - 1)
                back(bi - 1)
            front(bi)
        h2part(NB - 1)
        back(NB - 1)

    nc.finalize()
    return nc


_NC_CACHE = None


def _get_nc():
    global _NC_CACHE
    if _NC_CACHE is None:
        _NC_CACHE = build()
    return _NC_CACHE


def _hadamard128():
    h = np.array([[1.0]], dtype=np.float32)
    while h.shape[0] < 128:
        h = np.block([[h, h], [h, -h]])
    return h.astype(np.float32)


def make_in_maps(hidden_states, w_up, w_down):
    x = np.ascontiguousarray(hidden_states.reshape(TOKENS, H), dtype=np.float32)
    xT = np.ascontiguousarray(x.T)  # (H, TOKENS)

    s1 = np.float32(max(np.abs(w_up).mean(dtype=np.float32), np.float32(1e-5)))
    s2 = np.float32(max(np.abs(w_down).mean(dtype=np.float32), np.float32(1e-5)))
    tu = np.clip(np.round(w_up.astype(np.float32) / s1), -1.0, 1.0)
    td = np.clip(np.round(w_down.astype(np.float32) / s2), -1.0, 1.0)
    wu = np.ascontiguousarray(tu.T).astype(NP_FP8)   # (H, I)
    wd = np.ascontiguousarray(td.T).astype(NP_FP8)   # (I, H)
    ws = np.array([[s1, s2]], dtype=np.float32)
    h128 = _hadamard128()

    in_maps = []
    for c in range(NCORES):
        in_maps.append({
            "xT": np.ascontiguousarray(xT[:, c * T:(c + 1) * T]),
            "wu": wu,
            "wd": wd,
            "ws": ws,
            "h128": h128,
        })
    return in_maps


def kernel(hidden_states, w_up, w_down):
    nc = _get_nc()
    in_maps = make_in_maps(hidden_states, w_up, w_down)
    res = run_bass_kernel_spmd(nc, in_maps, list(range(NCORES))).results
    out = np.concatenate(
        [np.asarray(res[c]["out"], dtype=np.float32) for c in range(NCORES)], axis=0
    )
    return out.reshape(B, S, H)


# revision 18
# speedup vs baseline: 1.2082x; 1.2082x over previous
"""BitFeedForward (Hadamard + int8 act-quant + ternary weights) on 8 TRN2 cores.

Data-parallel over tokens (8192 tokens -> 1024 per core, 4 blocks of 256).
Weights are ternarized on the host (static packing: mean-abs scale + ternary
cast to fp8, transposed) so the device runs only the per-token path:
  FWHT = H128 on the PE (fp32r) + butterfly stages on DVE/GPSIMD (stage 1
  fused into the PSUM eviction), act quant -> int8-valued bf16 (exact),
  GEMMs bf16 x fp8 on the PE with exact integer arithmetic, per-token
  scales applied analytically at the end.
Weight matrices stream from DRAM in double-buffered slices; emission is a
2-stage software pipeline (layer-1+GEMM1 of block b+1 ahead of
layer-2+GEMM2 of block b) to keep the PE dense across the butterfly chains.
"""
import math
import numpy as np
import ml_dtypes
from contextlib import ExitStack

import concourse.bass as bass
from concourse import bacc
import concourse.tile as tile
import concourse.mybir as mybir
from concourse.bass_utils import run_bass_kernel_spmd
from concourse.masks import make_identity

F32 = mybir.dt.float32
F32R = mybir.dt.float32r
BF16 = mybir.dt.bfloat16
FP8 = mybir.dt.float8e4
NP_FP8 = ml_dtypes.float8_e4m3

NCORES = 8
B, S, H, I = 4, 2048, 2048, 4096
TOKENS = B * S            # 8192
T = TOKENS // NCORES      # 1024 tokens per core
TB = 256                  # tokens per block
NB = T // TB              # 4 blocks
NC1 = H // 128            # 16 k-chunks for layer 1
NC2 = I // 128            # 32 chunks for layer 2
CM = 12582912.0           # 1.5 * 2**23: fp32 add/sub rounds to nearest int
ISQ1 = 1.0 / math.sqrt(H)

ADD = mybir.AluOpType.add
SUB = mybir.AluOpType.subtract
MULT = mybir.AluOpType.mult
MAX = mybir.AluOpType.max
AF = mybir.ActivationFunctionType


def _bfly(eng, out_t, in_t, nchunk, sigma, span):
    """One FWHT butterfly stage over the chunk axis of [128, nchunk*span]."""
    iv = in_t[:].rearrange("p (g two s) -> p g two s", two=2, s=sigma * span)
    ov = out_t[:].rearrange("p (g two s) -> p g two s", two=2, s=sigma * span)
    assert iv.shape[1] == nchunk // (2 * sigma)
    eng.tensor_tensor(ov[:, :, 0, :], iv[:, :, 0, :], iv[:, :, 1, :], ADD)
    eng.tensor_tensor(ov[:, :, 1, :], iv[:, :, 0, :], iv[:, :, 1, :], SUB)


def build():
    nc = bacc.Bacc()
    x_in = nc.declare_dram_parameter("xT", [H, T], F32, isOutput=False)
    wu_in = nc.declare_dram_parameter("wu", [H, I], FP8, isOutput=False)
    wd_in = nc.declare_dram_parameter("wd", [I, H], FP8, isOutput=False)
    ws_in = nc.declare_dram_parameter("ws", [1, 2], F32, isOutput=False)
    h128_in = nc.declare_dram_parameter("h128", [128, 128], F32, isOutput=False)
    out_d = nc.declare_dram_parameter("out", [T, H], F32, isOutput=True)

    with tile.TileContext(nc) as tc, ExitStack() as ctx:
        const = ctx.enter_context(tc.tile_pool(name="const", bufs=1))
        t1p = ctx.enter_context(tc.tile_pool(name="t1p", bufs=2))
        t2p = ctx.enter_context(tc.tile_pool(name="t2p", bufs=2))
        xp = ctx.enter_context(tc.tile_pool(name="xp", bufs=2))
        l1p = ctx.enter_context(tc.tile_pool(name="l1p", bufs=1))
        q1p = ctx.enter_context(tc.tile_pool(name="q1p", bufs=1))
        rP = ctx.enter_context(tc.tile_pool(name="rP", bufs=1))
        l2p = ctx.enter_context(tc.tile_pool(name="l2p", bufs=1))
        q2p = ctx.enter_context(tc.tile_pool(name="q2p", bufs=1))
        rpp = ctx.enter_context(tc.tile_pool(name="rpp", bufs=1))
        shp = ctx.enter_context(tc.tile_pool(name="shp", bufs=2))
        outp = ctx.enter_context(tc.tile_pool(name="outp", bufs=1))
        med = ctx.enter_context(tc.tile_pool(name="med", bufs=1))
        tiny = ctx.enter_context(tc.tile_pool(name="tiny", bufs=1))
        tiny2 = ctx.enter_context(tc.tile_pool(name="tiny2", bufs=2))
        ps_h = ctx.enter_context(tc.tile_pool(name="ps_h", bufs=2, space="PSUM"))
        ps_a1 = ctx.enter_context(tc.tile_pool(name="ps_a1", bufs=2, space="PSUM"))
        ps_a2 = ctx.enter_context(tc.tile_pool(name="ps_a2", bufs=2, space="PSUM"))
        ps_sm = ctx.enter_context(tc.tile_pool(name="ps_sm", bufs=1, space="PSUM"))

        ident = const.tile([128, 128], F32)
        make_identity(nc, ident[:])
        h128 = const.tile([128, 128], F32)
        nc.sync.dma_start(h128[:], h128_in[:])
        h128r = const.tile([128, 128], F32R)
        nc.vector.tensor_copy(h128r[:], h128[:])
        ones_row = const.tile([1, 128], F32)
        nc.vector.memset(ones_row[:], 1.0)

        # broadcast [s1, s2] across partitions: sW[p, j] = s_j
        srow = const.tile([1, 2], F32)
        nc.sync.dma_start(srow[:], ws_in[:])
        psb = ps_sm.tile([128, 2], F32, tag="sm")
        nc.tensor.matmul(psb[:], ones_row[:], srow[:], start=True, stop=True)
        sW = const.tile([128, 2], F32)
        nc.vector.tensor_copy(sW[:], psb[:])

        st = [dict() for _ in range(NB)]  # per-block cross-phase tiles

        def h2part(bi):
            """H128 pass of layer 2 for block bi + fused sigma=1 butterfly."""
            d = st[bi]
            r = d["r"]
            vA = l2p.tile([128, NC2 * TB], F32, tag="ping")
            d["vA"] = vA
            for g in range(NC2 // 2):
                ph = ps_h.tile([128, 2 * TB], F32, tag="ph")
                for k in range(2):
                    m = 2 * g + k
                    nc.tensor.matmul(
                        ph[:, k * TB:(k + 1) * TB], h128r[:],
                        r[:, m * TB:(m + 1) * TB], start=True, stop=True)
                sh = shp.tile([128, TB], F32, tag="sh")
                nc.scalar.copy(sh[:], ph[:, 0:TB])
                nc.vector.tensor_tensor(
                    vA[:, (2 * g) * TB:(2 * g + 1) * TB],
                    sh[:], ph[:, TB:2 * TB], ADD)
                nc.vector.tensor_tensor(
                    vA[:, (2 * g + 1) * TB:(2 * g + 2) * TB],
                    sh[:], ph[:, TB:2 * TB], SUB)

        def front(bi):
            """x load, layer-1 FWHT + quant, GEMM1, relu^2 -> r (f32r)."""
            d = st[bi]
            tok0 = bi * TB

            # H128 pass of layer 1 with fused sigma=1 butterfly
            fA = l1p.tile([128, NC1 * TB], F32, tag="lp1")
            for g in range(NC1 // 2):
                xt = xp.tile([128, 2 * TB], F32, tag="x")
                nc.sync.dma_start(
                    xt[:].rearrange("p (c t) -> p c t", c=2),
                    x_in[g * 256:(g + 1) * 256, tok0:tok0 + TB].rearrange(
                        "(c p) t -> p c t", p=128),
                )
                ph = ps_h.tile([128, 2 * TB], F32, tag="ph")
                for k in range(2):
                    nc.tensor.matmul(
                        ph[:, k * TB:(k + 1) * TB], h128[:],
                        xt[:, k * TB:(k + 1) * TB], start=True, stop=True)
                sh = shp.tile([128, TB], F32, tag="sh")
                nc.scalar.copy(sh[:], ph[:, 0:TB])
                nc.vector.tensor_tensor(
                    fA[:, (2 * g) * TB:(2 * g + 1) * TB],
                    sh[:], ph[:, TB:2 * TB], ADD)
                nc.vector.tensor_tensor(
                    fA[:, (2 * g + 1) * TB:(2 * g + 2) * TB],
                    sh[:], ph[:, TB:2 * TB], SUB)
            fB = l1p.tile([128, NC1 * TB], F32, tag="lp2")
            _bfly(nc.gpsimd, fB, fA, NC1, 2, TB)
            fC = l1p.tile([128, NC1 * TB], F32, tag="lp1")
            _bfly(nc.vector, fC, fB, NC1, 4, TB)
            fD = l1p.tile([128, NC1 * TB], F32, tag="lp2")
            _bfly(nc.gpsimd, fD, fC, NC1, 8, TB)

            # per-token absmax -> scales (layer 1)
            P1 = med.tile([128, TB], F32, tag="p1")
            nc.vector.tensor_reduce(
                P1[:], fD[:].rearrange("p (c t) -> p t c", c=NC1),
                mybir.AxisListType.X, MAX, apply_absolute_value=True)
            pjt = ps_sm.tile([128, TB], F32, tag="sm")
            for j in range(2):
                nc.tensor.transpose(
                    pjt[:, j * 128:(j + 1) * 128], P1[:, j * 128:(j + 1) * 128],
                    ident[:])
            Mu = tiny.tile([128, 2], F32, tag="mu1")
            for j in range(2):
                nc.vector.tensor_reduce(
                    Mu[:, j:j + 1], pjt[:, j * 128:(j + 1) * 128],
                    mybir.AxisListType.X, MAX)
            M1 = tiny.tile([128, 2], F32, tag="m1")
            nc.vector.tensor_scalar(M1[:], Mu[:], ISQ1, 1e-5, MULT, MAX)
            rM1 = tiny.tile([128, 2], F32, tag="rm1")
            nc.vector.reciprocal(rM1[:], M1[:])
            s1t = tiny.tile([128, 2], F32, tag="s1t")
            nc.vector.tensor_scalar(s1t[:], rM1[:], 127.0 * ISQ1, None, MULT)
            ct = tiny.tile([128, 2], F32, tag="ct")
            nc.vector.tensor_tensor(ct[:], M1[:], sW[:, 0:1].broadcast_to([128, 2]), MULT)
            cc = tiny2.tile([128, 2], F32, tag="cc")
            nc.vector.tensor_tensor(cc[:], ct[:], ct[:], MULT)
            nc.vector.tensor_scalar(cc[:], cc[:], 1.0 / (127.0 * 127.0 * 64.0), None, MULT)
            d["cc"] = cc

            rows1 = []
            for j in range(2):
                stp = ps_sm.tile([1, 128], F32, tag="sm")
                nc.tensor.transpose(stp[:], s1t[:, j:j + 1], ident[:])
                rj = tiny.tile([1, 128], F32, tag=f"r1{j}")
                nc.vector.tensor_copy(rj[:], stp[:])
                rows1.append(rj)
            sbp = ps_sm.tile([128, TB], F32, tag="sm")
            for j in range(2):
                nc.tensor.matmul(
                    sbp[:, j * 128:(j + 1) * 128], ones_row[:], rows1[j][:],
                    start=True, stop=True)
            S1B = med.tile([128, TB], F32, tag="s1b")
            nc.vector.tensor_copy(S1B[:], sbp[:])

            # q1 = round(u * s1t), int8-valued bf16
            um = l1p.tile([128, NC1 * TB], F32, tag="lp1")
            nc.vector.tensor_tensor(
                um[:].rearrange("p (c t) -> p c t", c=NC1),
                fD[:].rearrange("p (c t) -> p c t", c=NC1),
                S1B[:, None, :].broadcast_to([128, NC1, TB]), MULT)
            q1t = q1p.tile([128, NC1 * TB], BF16, tag="q1")
            nc.vector.tensor_scalar(q1t[:], um[:], CM, CM, ADD, SUB)

            # GEMM1 + ReLU^2, streaming w_up^T in 512-col slices
            r = rP.tile([128, NC2 * TB], F32R, tag="r")
            d["r"] = r
            for s in range(I // 512):
                t1s = t1p.tile([128, NC1 * 512], FP8, tag="t1")
                nc.sync.dma_start(
                    t1s[:].rearrange("p (c o) -> p c o", c=NC1),
                    wu_in[:, s * 512:(s + 1) * 512].rearrange(
                        "(c p) o -> p c o", p=128),
                )
                for ocp in range(2):
                    acc = ps_a1.tile([128, 512], F32, tag="a1")
                    for half in range(2):
                        oc = ocp * 2 + half
                        for cp in range(NC1):
                            nc.tensor.matmul(
                                acc[:, half * TB:(half + 1) * TB],
                                t1s[:, cp * 512 + oc * 128: cp * 512 + (oc + 1) * 128],
                                q1t[:, cp * TB:(cp + 1) * TB],
                                start=(cp == 0), stop=(cp == NC1 - 1))
                    rp = rpp.tile([128, 512], F32, tag="rp")
                    nc.scalar.activation(rp[:], acc[:], AF.Relu, bias=0.0, scale=1.0)
                    m0 = s * 4 + ocp * 2
                    nc.scalar.activation(
                        r[:, m0 * TB:(m0 + 2) * TB], rp[:], AF.Square, bias=0.0)

        def back(bi):
            """layer-2 butterflies + quant, GEMM2, output."""
            d = st[bi]
            tok0 = bi * TB
            cc = d["cc"]
            vA = d["vA"]

            vB = l2p.tile([128, NC2 * TB], F32, tag="pong")
            _bfly(nc.gpsimd, vB, vA, NC2, 2, TB)
            vC = l2p.tile([128, NC2 * TB], F32, tag="ping")
            _bfly(nc.vector, vC, vB, NC2, 4, TB)
            vD = l2p.tile([128, NC2 * TB], F32, tag="pong")
            _bfly(nc.gpsimd, vD, vC, NC2, 8, TB)
            vE = l2p.tile([128, NC2 * TB], F32, tag="ping")
            _bfly(nc.vector, vE, vD, NC2, 16, TB)

            # per-token absmax + scales + quant, fully split by token halves
            # so GEMM2's first half starts as early as possible
            P2 = med.tile([128, TB], F32, tag="p2")
            M2 = tiny.tile([128, 2], F32, tag="m2")
            rM2 = tiny.tile([128, 2], F32, tag="rm2")
            s2t = tiny.tile([128, 2], F32, tag="s2t")
            f = tiny.tile([128, 2], F32, tag="f")
            Mu2 = tiny.tile([128, 2], F32, tag="mu2")
            S2B = med.tile([128, TB], F32, tag="s2b")
            vm = l2p.tile([128, NC2 * TB], F32, tag="pong")
            q2t = q2p.tile([128, NC2 * TB], BF16, tag="q2")
            for j in range(2):
                tj = slice(j * 128, (j + 1) * 128)
                jj = slice(j, j + 1)
                nc.vector.tensor_reduce(
                    P2[:, tj], vE[:].rearrange("p (m t) -> p t m", m=NC2)[:, tj, :],
                    mybir.AxisListType.X, MAX, apply_absolute_value=True)
                pjt2 = ps_sm.tile([128, 128], F32, tag="sm")
                nc.tensor.transpose(pjt2[:], P2[:, tj], ident[:])
                nc.vector.tensor_reduce(
                    Mu2[:, jj], pjt2[:], mybir.AxisListType.X, MAX)
                nc.vector.tensor_tensor(M2[:, jj], Mu2[:, jj], cc[:, jj], MULT)
                nc.vector.tensor_scalar(M2[:, jj], M2[:, jj], 1e-5, None, MAX)
                nc.vector.reciprocal(rM2[:, jj], M2[:, jj])
                nc.vector.tensor_tensor(s2t[:, jj], rM2[:, jj], cc[:, jj], MULT)
                nc.vector.tensor_scalar(s2t[:, jj], s2t[:, jj], 127.0, None, MULT)
                nc.vector.tensor_tensor(
                    f[:, jj], M2[:, jj], sW[:, 1:2], MULT)
                nc.vector.tensor_scalar(f[:, jj], f[:, jj], 1.0 / 127.0, None, MULT)
                stp = ps_sm.tile([1, 128], F32, tag="sm")
                nc.tensor.transpose(stp[:], s2t[:, jj], ident[:])
                rj = tiny.tile([1, 128], F32, tag=f"r2{j}")
                nc.vector.tensor_copy(rj[:], stp[:])
                sbp2 = ps_sm.tile([128, 128], F32, tag="sm")
                nc.tensor.matmul(sbp2[:], ones_row[:], rj[:], start=True, stop=True)
                nc.vector.tensor_copy(S2B[:, tj], sbp2[:])
                nc.vector.tensor_tensor(
                    vm[:].rearrange("p (m t) -> p m t", m=NC2)[:, :, tj],
                    vE[:].rearrange("p (m t) -> p m t", m=NC2)[:, :, tj],
                    S2B[:, None, tj].broadcast_to([128, NC2, 128]), MULT)
                nc.vector.tensor_scalar(
                    q2t[:].rearrange("p (m t) -> p m t", m=NC2)[:, :, tj],
                    vm[:].rearrange("p (m t) -> p m t", m=NC2)[:, :, tj],
                    CM, CM, ADD, SUB)

            # GEMM2 tokens-stationary, streaming w_down^T in 512-col slices
            for hs in range(4):
                halves = []
                for mh in range(2):
                    t2s = t2p.tile([128, 16 * 512], FP8, tag="t2")
                    nc.sync.dma_start(
                        t2s[:].rearrange("p (m h) -> p m h", m=16),
                        wd_in[mh * 2048:(mh + 1) * 2048,
                              hs * 512:(hs + 1) * 512].rearrange(
                            "(m p) h -> p m h", p=128),
                    )
                    halves.append(t2s)
                for j in range(2):
                    acc2 = ps_a2.tile([128, 512], F32, tag="a2")
                    for m2 in range(NC2):
                        t2s = halves[m2 // 16]
                        nc.tensor.matmul(
                            acc2[:],
                            q2t[:, m2 * TB + j * 128: m2 * TB + (j + 1) * 128],
                            t2s[:, (m2 % 16) * 512:((m2 % 16) + 1) * 512],
                            start=(m2 == 0), stop=(m2 == NC2 - 1))
                    ot = outp.tile([128, 512], F32, tag="ot")
                    nc.scalar.activation(
                        ot[:], acc2[:], AF.Identity, bias=0.0, scale=f[:, j:j + 1])
                    nc.sync.dma_start(
                        out_d[tok0 + j * 128: tok0 + (j + 1) * 128,
                              hs * 512:(hs + 1) * 512], ot[:])

        for bi in range(NB):
            if bi >= 1:
                h2part(bi - 1)
                back(bi - 1)
            front(bi)
        h2part(NB - 1)
        back(NB - 1)

    nc.finalize()
    return nc


_NC_CACHE = None


def _get_nc():
    global _NC_CACHE
    if _NC_CACHE is None:
        _NC_CACHE = build()
    return _NC_CACHE


def _hadamard128():
    h = np.array([[1.0]], dtype=np.float32)
    while h.shape[0] < 128:
        h = np.block([[h, h], [h, -h]])
    return h.astype(np.float32)


def make_in_maps(hidden_states, w_up, w_down):
    x = np.ascontiguousarray(hidden_states.reshape(TOKENS, H), dtype=np.float32)
    xT = np.ascontiguousarray(x.T)  # (H, TOKENS)

    s1 = np.float32(max(np.abs(w_up).mean(dtype=np.float32), np.float32(1e-5)))
    s2 = np.float32(max(np.abs(w_down).mean(dtype=np.float32), np.float32(1e-5)))
    tu = np.clip(np.round(w_up.astype(np.float32) / s1), -1.0, 1.0)
    td = np.clip(np.round(w_down.astype(np.float32) / s2), -1.0, 1.0)
    wu = np.ascontiguousarray(tu.T).astype(NP_FP8)   # (H, I)
    wd = np.ascontiguousarray(td.T).astype(NP_FP8)   # (I, H)
    ws = np.array([[s1, s2]], dtype=np.float32)
    h128 = _hadamard128()

    in_maps = []
    for c in range(NCORES):
        in_maps.append({
            "xT": np.ascontiguousarray(xT[:, c * T:(c + 1) * T]),
            "wu": wu,
            "wd": wd,
            "ws": ws,
            "h128": h128,
        })
    return in_maps


def kernel(hidden_states, w_up, w_down):
    nc = _get_nc()
    in_maps = make_in_maps(hidden_states, w_up, w_down)
    res = run_bass_kernel_spmd(nc, in_maps, list(range(NCORES))).results
    out = np.concatenate(
        [np.asarray(res[c]["out"], dtype=np.float32) for c in range(NCORES)], axis=0
    )
    return out.reshape(B, S, H)


# revision 22
# speedup vs baseline: 1.2276x; 1.0160x over previous
"""BitFeedForward (Hadamard + int8 act-quant + ternary weights) on 8 TRN2 cores.

Data-parallel over tokens (8192 tokens -> 1024 per core, 4 blocks of 256).
Weights are ternarized on the host (static packing: mean-abs scale + ternary
cast to fp8, transposed) so the device runs only the per-token path:
  FWHT = H128 on the PE (fp32r) + butterfly stages on DVE/GPSIMD (stage 1
  fused into the PSUM eviction), act quant -> int8-valued bf16 (exact),
  GEMMs bf16 x fp8 on the PE with exact integer arithmetic, per-token
  scales applied analytically at the end.
Weight matrices stream from DRAM in double-buffered slices; emission is a
2-stage software pipeline (layer-1+GEMM1 of block b+1 ahead of
layer-2+GEMM2 of block b) to keep the PE dense across the butterfly chains.
"""
import math
import numpy as np
import ml_dtypes
from contextlib import ExitStack

import concourse.bass as bass
from concourse import bacc
import concourse.tile as tile
import concourse.mybir as mybir
from concourse.bass_utils import run_bass_kernel_spmd
from concourse.masks import make_identity

F32 = mybir.dt.float32
F32R = mybir.dt.float32r
BF16 = mybir.dt.bfloat16
FP8 = mybir.dt.float8e4
NP_FP8 = ml_dtypes.float8_e4m3

NCORES = 8
B, S, H, I = 4, 2048, 2048, 4096
TOKENS = B * S            # 8192
T = TOKENS // NCORES      # 1024 tokens per core
TB = 256                  # tokens per block
NB = T // TB              # 4 blocks
NC1 = H // 128            # 16 k-chunks for layer 1
NC2 = I // 128            # 32 chunks for layer 2
CM = 12582912.0           # 1.5 * 2**23: fp32 add/sub rounds to nearest int
ISQ1 = 1.0 / math.sqrt(H)

ADD = mybir.AluOpType.add
SUB = mybir.AluOpType.subtract
MULT = mybir.AluOpType.mult
MAX = mybir.AluOpType.max
AF = mybir.ActivationFunctionType


def _bfly(eng, out_t, in_t, nchunk, sigma, span):
    """One FWHT butterfly stage over the chunk axis of [128, nchunk*span]."""
    iv = in_t[:].rearrange("p (g two s) -> p g two s", two=2, s=sigma * span)
    ov = out_t[:].rearrange("p (g two s) -> p g two s", two=2, s=sigma * span)
    assert iv.shape[1] == nchunk // (2 * sigma)
    eng.tensor_tensor(ov[:, :, 0, :], iv[:, :, 0, :], iv[:, :, 1, :], ADD)
    eng.tensor_tensor(ov[:, :, 1, :], iv[:, :, 0, :], iv[:, :, 1, :], SUB)


def build():
    nc = bacc.Bacc()
    x_in = nc.declare_dram_parameter("xT", [H, T], F32, isOutput=False)
    wu_in = nc.declare_dram_parameter("wu", [H, I], FP8, isOutput=False)
    wd_in = nc.declare_dram_parameter("wd", [I, H], FP8, isOutput=False)
    ws_in = nc.declare_dram_parameter("ws", [1, 2], F32, isOutput=False)
    h128_in = nc.declare_dram_parameter("h128", [128, 128], F32, isOutput=False)
    out_d = nc.declare_dram_parameter("out", [T, H], F32, isOutput=True)

    with tile.TileContext(nc) as tc, ExitStack() as ctx:
        const = ctx.enter_context(tc.tile_pool(name="const", bufs=1))
        t1p = ctx.enter_context(tc.tile_pool(name="t1p", bufs=2))
        t2p = ctx.enter_context(tc.tile_pool(name="t2p", bufs=2))
        xp = ctx.enter_context(tc.tile_pool(name="xp", bufs=2))
        l1p = ctx.enter_context(tc.tile_pool(name="l1p", bufs=1))
        q1p = ctx.enter_context(tc.tile_pool(name="q1p", bufs=1))
        rP = ctx.enter_context(tc.tile_pool(name="rP", bufs=1))
        l2p = ctx.enter_context(tc.tile_pool(name="l2p", bufs=1))
        q2p = ctx.enter_context(tc.tile_pool(name="q2p", bufs=1))
        rpp = ctx.enter_context(tc.tile_pool(name="rpp", bufs=1))
        shp = ctx.enter_context(tc.tile_pool(name="shp", bufs=2))
        outp = ctx.enter_context(tc.tile_pool(name="outp", bufs=1))
        med = ctx.enter_context(tc.tile_pool(name="med", bufs=1))
        tiny = ctx.enter_context(tc.tile_pool(name="tiny", bufs=1))
        tiny2 = ctx.enter_context(tc.tile_pool(name="tiny2", bufs=2))
        ps_h = ctx.enter_context(tc.tile_pool(name="ps_h", bufs=2, space="PSUM"))
        ps_a1 = ctx.enter_context(tc.tile_pool(name="ps_a1", bufs=2, space="PSUM"))
        ps_a2 = ctx.enter_context(tc.tile_pool(name="ps_a2", bufs=1, space="PSUM"))
        ps_sm = ctx.enter_context(tc.tile_pool(name="ps_sm", bufs=1, space="PSUM"))

        ident = const.tile([128, 128], F32)
        make_identity(nc, ident[:])
        h128 = const.tile([128, 128], F32)
        nc.sync.dma_start(h128[:], h128_in[:])
        h128r = const.tile([128, 128], F32R)
        nc.vector.tensor_copy(h128r[:], h128[:])
        ones_row = const.tile([1, 128], F32)
        nc.vector.memset(ones_row[:], 1.0)

        # broadcast [s1, s2] across partitions: sW[p, j] = s_j
        srow = const.tile([1, 2], F32)
        nc.sync.dma_start(srow[:], ws_in[:])
        psb = ps_sm.tile([128, 2], F32, tag="sm")
        nc.tensor.matmul(psb[:], ones_row[:], srow[:], start=True, stop=True)
        sW = const.tile([128, 2], F32)
        nc.vector.tensor_copy(sW[:], psb[:])

        st = [dict() for _ in range(NB)]  # per-block cross-phase tiles

        def h2part(bi):
            """H128 pass of layer 2 for block bi + fused sigma=1 butterfly."""
            d = st[bi]
            r = d["r"]
            vA = l2p.tile([128, NC2 * TB], F32, tag="ping")
            d["vA"] = vA
            for g in range(NC2 // 2):
                ph = ps_h.tile([128, 2 * TB], F32, tag="ph")
                for k in range(2):
                    m = 2 * g + k
                    nc.tensor.matmul(
                        ph[:, k * TB:(k + 1) * TB], h128r[:],
                        r[:, m * TB:(m + 1) * TB], start=True, stop=True)
                sh = shp.tile([128, TB], F32, tag="sh")
                nc.scalar.copy(sh[:], ph[:, 0:TB])
                nc.vector.tensor_tensor(
                    vA[:, (2 * g) * TB:(2 * g + 1) * TB],
                    sh[:], ph[:, TB:2 * TB], ADD)
                nc.vector.tensor_tensor(
                    vA[:, (2 * g + 1) * TB:(2 * g + 2) * TB],
                    sh[:], ph[:, TB:2 * TB], SUB)

        def front(bi):
            """x load, layer-1 FWHT + quant, GEMM1, relu^2 -> r (f32r)."""
            d = st[bi]
            tok0 = bi * TB

            # H128 pass of layer 1 with fused sigma=1 butterfly
            fA = l1p.tile([128, NC1 * TB], F32, tag="lp1")
            for g in range(NC1 // 2):
                xt = xp.tile([128, 2 * TB], F32, tag="x")
                nc.sync.dma_start(
                    xt[:].rearrange("p (c t) -> p c t", c=2),
                    x_in[g * 256:(g + 1) * 256, tok0:tok0 + TB].rearrange(
                        "(c p) t -> p c t", p=128),
                )
                ph = ps_h.tile([128, 2 * TB], F32, tag="ph1")
                for k in range(2):
                    nc.tensor.matmul(
                        ph[:, k * TB:(k + 1) * TB], h128[:],
                        xt[:, k * TB:(k + 1) * TB], start=True, stop=True)
                sh = shp.tile([128, TB], F32, tag="sh")
                nc.scalar.copy(sh[:], ph[:, 0:TB])
                nc.vector.tensor_tensor(
                    fA[:, (2 * g) * TB:(2 * g + 1) * TB],
                    sh[:], ph[:, TB:2 * TB], ADD)
                nc.vector.tensor_tensor(
                    fA[:, (2 * g + 1) * TB:(2 * g + 2) * TB],
                    sh[:], ph[:, TB:2 * TB], SUB)
            fB = l1p.tile([128, NC1 * TB], F32, tag="lp2")
            _bfly(nc.gpsimd, fB, fA, NC1, 2, TB)
            fC = l1p.tile([128, NC1 * TB], F32, tag="lp1")
            _bfly(nc.vector, fC, fB, NC1, 4, TB)
            fD = l1p.tile([128, NC1 * TB], F32, tag="lp2")
            _bfly(nc.gpsimd, fD, fC, NC1, 8, TB)

            # per-token absmax -> scales (layer 1), split by token halves
            P1 = med.tile([128, TB], F32, tag="p1")
            Mu = tiny.tile([128, 2], F32, tag="mu1")
            M1 = tiny.tile([128, 2], F32, tag="m1")
            rM1 = tiny.tile([128, 2], F32, tag="rm1")
            s1t = tiny.tile([128, 2], F32, tag="s1t")
            ct = tiny.tile([128, 2], F32, tag="ct")
            cc = tiny2.tile([128, 2], F32, tag="cc")
            S1B = med.tile([128, TB], F32, tag="s1b")
            for j in range(2):
                tj = slice(j * 128, (j + 1) * 128)
                jj = slice(j, j + 1)
                nc.vector.tensor_reduce(
                    P1[:, tj], fD[:].rearrange("p (c t) -> p t c", c=NC1)[:, tj, :],
                    mybir.AxisListType.X, MAX, apply_absolute_value=True)
                pjt = ps_sm.tile([128, 128], F32, tag="sm")
                nc.tensor.transpose(pjt[:], P1[:, tj], ident[:])
                nc.vector.tensor_reduce(
                    Mu[:, jj], pjt[:], mybir.AxisListType.X, MAX)
                nc.vector.tensor_scalar(M1[:, jj], Mu[:, jj], ISQ1, 1e-5, MULT, MAX)
                nc.vector.reciprocal(rM1[:, jj], M1[:, jj])
                nc.vector.tensor_scalar(s1t[:, jj], rM1[:, jj], 127.0 * ISQ1, None, MULT)
                nc.vector.tensor_tensor(ct[:, jj], M1[:, jj], sW[:, 0:1], MULT)
                nc.vector.tensor_tensor(cc[:, jj], ct[:, jj], ct[:, jj], MULT)
                nc.vector.tensor_scalar(cc[:, jj], cc[:, jj], 1.0 / (127.0 * 127.0 * 64.0), None, MULT)
                stp = ps_sm.tile([1, 128], F32, tag="sm")
                nc.tensor.transpose(stp[:], s1t[:, jj], ident[:])
                rj = tiny.tile([1, 128], F32, tag=f"r1{j}")
                nc.vector.tensor_copy(rj[:], stp[:])
                sbp = ps_sm.tile([128, 128], F32, tag="sm")
                nc.tensor.matmul(sbp[:], ones_row[:], rj[:], start=True, stop=True)
                nc.vector.tensor_copy(S1B[:, tj], sbp[:])
            d["cc"] = cc

            # q1 = round(u * s1t), int8-valued bf16
            um = l1p.tile([128, NC1 * TB], F32, tag="lp1")
            nc.vector.tensor_tensor(
                um[:].rearrange("p (c t) -> p c t", c=NC1),
                fD[:].rearrange("p (c t) -> p c t", c=NC1),
                S1B[:, None, :].broadcast_to([128, NC1, TB]), MULT)
            q1t = q1p.tile([128, NC1 * TB], BF16, tag="q1")
            nc.vector.tensor_scalar(q1t[:], um[:], CM, CM, ADD, SUB)

            # GEMM1 + ReLU^2, streaming w_up^T in 512-col slices
            r = rP.tile([128, NC2 * TB], F32R, tag="r")
            d["r"] = r
            for s in range(I // 512):
                t1s = t1p.tile([128, NC1 * 512], FP8, tag="t1")
                nc.sync.dma_start(
                    t1s[:].rearrange("p (c o) -> p c o", c=NC1),
                    wu_in[:, s * 512:(s + 1) * 512].rearrange(
                        "(c p) o -> p c o", p=128),
                )
                for ocp in range(2):
                    acc = ps_a1.tile([128, 512], F32, tag="a1")
                    for half in range(2):
                        oc = ocp * 2 + half
                        for cp in range(NC1):
                            nc.tensor.matmul(
                                acc[:, half * TB:(half + 1) * TB],
                                t1s[:, cp * 512 + oc * 128: cp * 512 + (oc + 1) * 128],
                                q1t[:, cp * TB:(cp + 1) * TB],
                                start=(cp == 0), stop=(cp == NC1 - 1))
                    rp = rpp.tile([128, 512], F32, tag="rp")
                    nc.scalar.activation(rp[:], acc[:], AF.Relu, bias=0.0, scale=1.0)
                    m0 = s * 4 + ocp * 2
                    nc.scalar.activation(
                        r[:, m0 * TB:(m0 + 2) * TB], rp[:], AF.Square, bias=0.0)

        def back(bi):
            """layer-2 butterflies + quant, GEMM2, output."""
            d = st[bi]
            tok0 = bi * TB
            cc = d["cc"]
            vA = d["vA"]

            vB = l2p.tile([128, NC2 * TB], F32, tag="pong")
            _bfly(nc.gpsimd, vB, vA, NC2, 2, TB)
            vC = l2p.tile([128, NC2 * TB], F32, tag="ping")
            _bfly(nc.vector, vC, vB, NC2, 4, TB)
            vD = l2p.tile([128, NC2 * TB], F32, tag="pong")
            _bfly(nc.gpsimd, vD, vC, NC2, 8, TB)
            vE = l2p.tile([128, NC2 * TB], F32, tag="ping")
            _bfly(nc.vector, vE, vD, NC2, 16, TB)

            # per-token absmax + scales + quant, fully split by token halves
            # so GEMM2's first half starts as early as possible
            P2 = med.tile([128, TB], F32, tag="p2")
            M2 = tiny.tile([128, 2], F32, tag="m2")
            rM2 = tiny.tile([128, 2], F32, tag="rm2")
            s2t = tiny.tile([128, 2], F32, tag="s2t")
            f = tiny.tile([128, 2], F32, tag="f")
            Mu2 = tiny.tile([128, 2], F32, tag="mu2")
            S2B = med.tile([128, TB], F32, tag="s2b")
            vm = l2p.tile([128, NC2 * TB], F32, tag="pong")
            q2t = q2p.tile([128, NC2 * TB], BF16, tag="q2")
            for j in range(2):
                tj = slice(j * 128, (j + 1) * 128)
                jj = slice(j, j + 1)
                nc.vector.tensor_reduce(
                    P2[:, tj], vE[:].rearrange("p (m t) -> p t m", m=NC2)[:, tj, :],
                    mybir.AxisListType.X, MAX, apply_absolute_value=True)
                pjt2 = ps_sm.tile([128, 128], F32, tag="sm2")
                nc.tensor.transpose(pjt2[:], P2[:, tj], ident[:])
                nc.vector.tensor_reduce(
                    Mu2[:, jj], pjt2[:], mybir.AxisListType.X, MAX)
                nc.vector.tensor_tensor(M2[:, jj], Mu2[:, jj], cc[:, jj], MULT)
                nc.vector.tensor_scalar(M2[:, jj], M2[:, jj], 1e-5, None, MAX)
                nc.vector.reciprocal(rM2[:, jj], M2[:, jj])
                nc.vector.tensor_tensor(s2t[:, jj], rM2[:, jj], cc[:, jj], MULT)
                nc.vector.tensor_scalar(s2t[:, jj], s2t[:, jj], 127.0, None, MULT)
                nc.vector.tensor_tensor(
                    f[:, jj], M2[:, jj], sW[:, 1:2], MULT)
                nc.vector.tensor_scalar(f[:, jj], f[:, jj], 1.0 / 127.0, None, MULT)
                stp = ps_sm.tile([1, 128], F32, tag="sm2")
                nc.tensor.transpose(stp[:], s2t[:, jj], ident[:])
                rj = tiny.tile([1, 128], F32, tag=f"r2{j}")
                nc.vector.tensor_copy(rj[:], stp[:])
                sbp2 = ps_sm.tile([128, 128], F32, tag="sm2")
                nc.tensor.matmul(sbp2[:], ones_row[:], rj[:], start=True, stop=True)
                nc.vector.tensor_copy(S2B[:, tj], sbp2[:])
                nc.vector.tensor_tensor(
                    vm[:].rearrange("p (m t) -> p m t", m=NC2)[:, :, tj],
                    vE[:].rearrange("p (m t) -> p m t", m=NC2)[:, :, tj],
                    S2B[:, None, tj].broadcast_to([128, NC2, 128]), MULT)
                nc.vector.tensor_scalar(
                    q2t[:].rearrange("p (m t) -> p m t", m=NC2)[:, :, tj],
                    vm[:].rearrange("p (m t) -> p m t", m=NC2)[:, :, tj],
                    CM, CM, ADD, SUB)

            # GEMM2 tokens-stationary, streaming w_down^T in 512-col slices
            for hs in range(4):
                halves = []
                for mh in range(2):
                    t2s = t2p.tile([128, 16 * 512], FP8, tag="t2")
                    nc.sync.dma_start(
                        t2s[:].rearrange("p (m h) -> p m h", m=16),
                        wd_in[mh * 2048:(mh + 1) * 2048,
                              hs * 512:(hs + 1) * 512].rearrange(
                            "(m p) h -> p m h", p=128),
                    )
                    halves.append(t2s)
                for j in range(2):
                    acc2 = ps_a2.tile([128, 512], F32, tag="a2")
                    for m2 in range(NC2):
                        t2s = halves[m2 // 16]
                        nc.tensor.matmul(
                            acc2[:],
                            q2t[:, m2 * TB + j * 128: m2 * TB + (j + 1) * 128],
                            t2s[:, (m2 % 16) * 512:((m2 % 16) + 1) * 512],
                            start=(m2 == 0), stop=(m2 == NC2 - 1))
                    ot = outp.tile([128, 512], F32, tag="ot")
                    nc.scalar.activation(
                        ot[:], acc2[:], AF.Identity, bias=0.0, scale=f[:, j:j + 1])
                    nc.sync.dma_start(
                        out_d[tok0 + j * 128: tok0 + (j + 1) * 128,
                              hs * 512:(hs + 1) * 512], ot[:])

        for bi in range(NB):
            if bi >= 1:
                h2part(bi - 1)
                back(bi - 1)
            front(bi)
        h2part(NB - 1)
        back(NB - 1)

    nc.finalize()
    return nc


_NC_CACHE = None


def _get_nc():
    global _NC_CACHE
    if _NC_CACHE is None:
        _NC_CACHE = build()
    return _NC_CACHE


def _hadamard128():
    h = np.array([[1.0]], dtype=np.float32)
    while h.shape[0] < 128:
        h = np.block([[h, h], [h, -h]])
    return h.astype(np.float32)


def make_in_maps(hidden_states, w_up, w_down):
    x = np.ascontiguousarray(hidden_states.reshape(TOKENS, H), dtype=np.float32)
    xT = np.ascontiguousarray(x.T)  # (H, TOKENS)

    s1 = np.float32(max(np.abs(w_up).mean(dtype=np.float32), np.float32(1e-5)))
    s2 = np.float32(max(np.abs(w_down).mean(dtype=np.float32), np.float32(1e-5)))
    tu = np.clip(np.round(w_up.astype(np.float32) / s1), -1.0, 1.0)
    td = np.clip(np.round(w_down.astype(np.float32) / s2), -1.0, 1.0)
    wu = np.ascontiguousarray(tu.T).astype(NP_FP8)   # (H, I)
    wd = np.ascontiguousarray(td.T).astype(NP_FP8)   # (I, H)
    ws = np.array([[s1, s2]], dtype=np.float32)
    h128 = _hadamard128()

    in_maps = []
    for c in range(NCORES):
        in_maps.append({
            "xT": np.ascontiguousarray(xT[:, c * T:(c + 1) * T]),
            "wu": wu,
            "wd": wd,
            "ws": ws,
            "h128": h128,
        })
    return in_maps


def kernel(hidden_states, w_up, w_down):
    nc = _get_nc()
    in_maps = make_in_maps(hidden_states, w_up, w_down)
    res = run_bass_kernel_spmd(nc, in_maps, list(range(NCORES))).results
    out = np.concatenate(
        [np.asarray(res[c]["out"], dtype=np.float32) for c in range(NCORES)], axis=0
    )
    return out.reshape(B, S, H)
